# revision 1
# baseline (speedup 1.0000x reference)
"""GAT (5-layer, dense-adjacency) Trainium2 kernel, sharded across 8 NeuronCores.

Sharding: query-node rows split 512/core. Each core holds its transposed
additive attention mask [4096(j), 512(i)] resident in SBUF, computes the
full Wh per layer (cheap), its row-block of attention via a fused custom
DVE op (leaky(s+d)+addmask) + ACT exp + bf16 matmul with a ones-column for
softmax denominators, then AllGathers the transposed activations.
"""

import numpy as np

import concourse.bacc as bacc
import concourse.mybir as mybir
import concourse.tile as tile
from concourse.bass_utils import run_bass_kernel_spmd

import concourse.dve_ops as dve_ops
from concourse.dve_spec import Spec, Src0, Src1, C0, C1, maxx, lower
from concourse.dve_spec import _has_src1 as _spec_has_src1
from concourse.dve_uop import DveOpSpec

try:
    import ml_dtypes

    _BF16 = ml_dtypes.bfloat16
except ImportError:  # pragma: no cover
    _BF16 = np.float32

dt = mybir.dt
AF = mybir.ActivationFunctionType

# ---------------------------------------------------------------- constants
N = 4096
NCORE = 8
ROWS = N // NCORE  # 512 query rows per core
P = 128
JT = N // P  # 32 j-tiles
NEG = -30000.0  # additive mask for non-edges; exp(x-30000) == 0 in f32
ALPHA = 0.1
# (fin, fout, heads, concat, elu_after)
CFG = [
    (256, 128, 8, True, True),
    (128, 64, 8, True, True),
    (64, 32, 4, True, True),
    (32, 16, 1, True, False),
    (16, 8, 1, False, False),
]

# ---------------------------------------------------------------- custom op
LEAKY_BIAS_ADDMASK = dve_ops.DveOp(
    "LEAKY_BIAS_ADDMASK",
    Spec(
        body=maxx(Src0 + C0, (Src0 + C0) * C1) + Src1,
        reference=lambda in0, in1, s0, s1, imm2: (
            np.maximum(in0 + s0, (in0 + s0) * s1) + in1
        ).astype(np.float32),
    ),
    subdim=False,
    uops_sha={},
)


def _register_custom_op(op):
    if op.name in dve_ops._SUB_OPCODE_FOR_NAME:
        return
    idx = dve_ops._CUSTOM_DVE_ROW_BASE + len(dve_ops.OPS)
    assert idx < 0x20
    dve_ops.OPS.append(op)
    dve_ops.CUSTOM_DVE_SPECS[op.name] = op.spec
    dve_ops._SUB_OPCODE_FOR_NAME[op.name] = idx
    shas = {}
    for ver in ("v3", "v4"):
        try:
            s = DveOpSpec(
                name=op.name,
                opcode=idx,
                uops=lower(op.spec, ver=ver),
                rd1_en=_spec_has_src1(op.spec),
            )
            shas[ver] = s.sha(ver)
        except Exception:
            pass
    object.__setattr__(op, "uops_sha", shas)


_register_custom_op(LEAKY_BIAS_ADDMASK)


# ---------------------------------------------------------------- builder
def build_kernel():
    import os as _os
    debug_taps = bool(_os.environ.get("DEBUG_TAPS"))
    nc = bacc.Bacc("TRN2", target_bir_lowering=False, debug=False)

    adjrows = nc.dram_tensor("adjrows", [ROWS, N], dt.int32, kind="ExternalInput")
    x0T_own = nc.dram_tensor("x0T_own", [256, ROWS], dt.float32, kind="ExternalInput")
    wext_dram = {}
    ws_dram = {}
    for li, (fin, fout, h, concat, _elu) in enumerate(CFG, start=1):
        dh = fout // h if concat else fout
        wext_dram[li] = nc.dram_tensor(
            f"wext{li}", [fin, h * dh + h], dt.float32, kind="ExternalInput"
        )
        ws_dram[li] = nc.dram_tensor(f"ws{li}", [fin, h], dt.float32, kind="ExternalInput")

    pool_out = nc.dram_tensor("pool_part", [8, 1], dt.float32, kind="ExternalOutput")
    if debug_taps:
        dbg_d = nc.dram_tensor("dbg_d", [P, JT, 8], dt.float32, kind="ExternalOutput")
        dbg_wh = nc.dram_tensor("dbg_wh", [P, 8 * 33], dt.float32, kind="ExternalOutput")
        dbg_x = {}
        for _li, (_f, _fo, _h, _c, _e) in enumerate(CFG, start=1):
            _fo2 = _fo if _c else _fo
            dbg_x[_li] = nc.dram_tensor(f"dbg_x{_li}", [_fo2, ROWS], dt.float32, kind="ExternalOutput")

    ident_np = np.eye(P, dtype=_BF16)
    ident_dram = nc.inline_tensor(ident_np, name="ident128")

    with tile.TileContext(nc) as tc:
        with (
            tc.tile_pool(name="persist", bufs=1) as persist,
            tc.tile_pool(name="dram", bufs=1, space="DRAM") as drampool,
            tc.tile_pool(name="xTown", bufs=3) as xTown_pool,
            tc.tile_pool(name="layerbuf", bufs=1) as layerbuf,
            tc.tile_pool(name="work", bufs=2) as work,
            tc.tile_pool(name="srep", bufs=4) as srep_pool,
            tc.tile_pool(name="small", bufs=2) as small,
            tc.tile_pool(name="whps", bufs=2, space="PSUM") as whps,
            tc.tile_pool(name="sps", bufs=1, space="PSUM") as sps,
            tc.tile_pool(name="attps", bufs=4, space="PSUM") as attps,
        ):
            # ---------------- persistent tiles
            maskT = persist.tile([P, JT, ROWS], dt.float32, tag="maskT")
            ident_sb = persist.tile([P, P], dt.bfloat16, tag="ident")
            nc.sync.dma_start(ident_sb[:], ident_dram[:])
            ones_row = persist.tile([1, P], dt.float32, tag="ones_row")
            nc.vector.memset(ones_row[:], 1.0)
            negb = persist.tile([P, 1], dt.float32, tag="negb")
            nc.vector.memset(negb[:], NEG)

            wext_sb = {}
            ws_sb = {}
            for li, (fin, fout, h, concat, _elu) in enumerate(CFG, start=1):
                dh = fout // h if concat else fout
                nft = (fin + P - 1) // P
                wext_sb[li] = []
                ws_sb[li] = []
                for ft in range(nft):
                    fr = min(P, fin - ft * P)
                    wt = persist.tile([fr, h * dh + h], dt.float32, tag=f"wext{li}_{ft}")
                    nc.sync.dma_start(wt[:], wext_dram[li][ft * P : ft * P + fr, :])
                    wext_sb[li].append(wt)
                    st = persist.tile([fr, h], dt.float32, tag=f"ws{li}_{ft}")
                    nc.sync.dma_start(st[:], ws_dram[li][ft * P : ft * P + fr, :])
                    ws_sb[li].append(st)

            # ---------------- mask build (transpose adj rows -> additive maskT)
            CH = 1024
            for c0 in range(0, N, CH):
                for ib in range(ROWS // P):
                    stage_i = work.tile([P, CH], dt.int32, tag="stage_i")
                    nc.sync.dma_start(
                        stage_i[:], adjrows[ib * P : (ib + 1) * P, c0 : c0 + CH]
                    )
                    stage_b = work.tile([P, CH], dt.bfloat16, tag="stage_b")
                    nc.gpsimd.tensor_copy(stage_b[:], stage_i[:])
                    for k in range(CH // P):
                        jt = (c0 + k * P) // P
                        tps = sps.tile([P, P], dt.bfloat16, tag="ps_row")
                        nc.tensor.transpose(
                            tps[:], stage_b[:, k * P : (k + 1) * P], ident_sb[:]
                        )
                        nc.scalar.activation(
                            maskT[:, jt, ib * P : (ib + 1) * P],
                            tps[:],
                            AF.Identity,
                            bias=negb[:],
                            scale=-NEG,
                        )

            # ---------------- L1 own activations from input
            xTown_cur = []
            for ft in range(2):
                to = xTown_pool.tile([P, ROWS], dt.float32, tag="xTown")
                nc.sync.dma_start(to[:], x0T_own[ft * P : (ft + 1) * P, :])
                xTown_cur.append(to)

            for li, (fin, fout, h, concat, elu) in enumerate(CFG, start=1):
                dh = fout // h if concat else fout
                hdh = h * dh
                CW = hdh + h  # compact row width: Wh values + d column
                nft = (fin + P - 1) // P
                is_last = li == len(CFG)

                # ---- (A) own-block Wh (+d) for the 4 own j-chunks
                own_hi = work.tile([P, 4, CW], dt.bfloat16, tag="own_hi")
                own_lo = work.tile([P, 4, CW], dt.bfloat16, tag="own_lo")
                for k in range(4):
                    pw = whps.tile([P, CW], dt.float32, tag="pw")
                    for ft in range(nft):
                        fr = min(P, fin - ft * P)
                        nc.tensor.matmul(
                            pw[:],
                            xTown_cur[ft][0:fr, k * P : (k + 1) * P],
                            wext_sb[li][ft][:],
                            start=(ft == 0),
                            stop=(ft == nft - 1),
                        )
                    nc.scalar.copy(own_hi[:, k, :], pw[:])
                    nc.vector.tensor_sub(own_lo[:, k, :], pw[:], own_hi[:, k, :])

                # ---- (B) s_rep per head (from own activations)
                sreps = []
                for hh in range(h):
                    ps_row = sps.tile([1, ROWS], dt.float32, tag="ps_row")
                    for ft in range(nft):
                        fr = min(P, fin - ft * P)
                        nc.tensor.matmul(
                            ps_row[:],
                            ws_sb[li][ft][:, hh : hh + 1],
                            xTown_cur[ft][0:fr, :],
                            start=(ft == 0),
                            stop=(ft == nft - 1),
                        )
                    s_row = small.tile([1, ROWS], dt.float32, tag="vec1")
                    nc.vector.tensor_copy(s_row[:], ps_row[:])
                    ps_rep = sps.tile([P, ROWS], dt.float32, tag="ps_rep")
                    nc.tensor.matmul(
                        ps_rep[:], ones_row[:], s_row[:], start=True, stop=True
                    )
                    srt = srep_pool.tile([P, ROWS], dt.float32, tag="srep")
                    nc.scalar.copy(srt[:], ps_rep[:])
                    sreps.append(srt)

                # ---- (C) pack + AllGather: A = headsA values + d, B = headsB
                hA = (h // 2 if h > 1 else 1) if not _os.environ.get("NOSPLIT") else h
                hB = h - hA
                CWa = hA * dh + h
                CWb = hB * dh
                ag_a_in = drampool.tile([2, 4 * P, CWa], dt.bfloat16, tag=f"again{li}")
                ag_a_out = drampool.tile(
                    [NCORE, 2, 4 * P, CWa], dt.bfloat16, tag=f"agaout{li}"
                )
                nc.sync.dma_start(
                    ag_a_in[0].rearrange("(k p) c -> p k c", p=P), own_hi[:, :, 0:CWa]
                )
                nc.sync.dma_start(
                    ag_a_in[1].rearrange("(k p) c -> p k c", p=P), own_lo[:, :, 0:CWa]
                )
                nc.gpsimd.collective_compute(
                    "AllGather",
                    mybir.AluOpType.bypass,
                    replica_groups=[list(range(NCORE))],
                    ins=[ag_a_in.opt()],
                    outs=[ag_a_out.opt()],
                )
                if hB:
                    ag_b_in = drampool.tile(
                        [2, 4 * P, CWb], dt.bfloat16, tag=f"agbin{li}"
                    )
                    ag_b_out = drampool.tile(
                        [NCORE, 2, 4 * P, CWb], dt.bfloat16, tag=f"agbout{li}"
                    )
                    nc.sync.dma_start(
                        ag_b_in[0].rearrange("(k p) c -> p k c", p=P),
                        own_hi[:, :, CWa:CW],
                    )
                    nc.sync.dma_start(
                        ag_b_in[1].rearrange("(k p) c -> p k c", p=P),
                        own_lo[:, :, CWa:CW],
                    )
                    nc.gpsimd.collective_compute(
                        "AllGather",
                        mybir.AluOpType.bypass,
                        replica_groups=[list(range(NCORE))],
                        ins=[ag_b_in.opt()],
                        outs=[ag_b_out.opt()],
                    )

                # ---- (D/E) load + unpack into padded matmul layout
                whrow = layerbuf.tile([P, JT, h * 33], dt.bfloat16, tag="whrow")
                whrow_lo = layerbuf.tile([P, JT, h * 33], dt.bfloat16, tag="whrow_lo")
                d_sb = layerbuf.tile([P, JT, h], dt.float32, tag="d_sb")
                wh4 = whrow.rearrange("p j (a b) -> p a j b", a=h)
                wl4 = whrow_lo.rearrange("p j (a b) -> p a j b", a=h)
                for hh in range(h):
                    nc.gpsimd.memset(wh4[:, hh, :, dh:33], 0.0)
                    nc.gpsimd.memset(wl4[:, hh, :, dh:33], 0.0)
                    nc.gpsimd.memset(wh4[:, hh, :, 32:33], 1.0)

                cmp_a_hi = layerbuf.tile([P, JT, CWa], dt.bfloat16, tag="cmp_a_hi")
                cmp_a_lo = layerbuf.tile([P, JT, CWa], dt.bfloat16, tag="cmp_a_lo")
                for r in range(NCORE):
                    nc.sync.dma_start(
                        cmp_a_hi[:, 4 * r : 4 * (r + 1), :],
                        ag_a_out[r, 0].rearrange("(k p) c -> p k c", p=P),
                    )
                    nc.sync.dma_start(
                        cmp_a_lo[:, 4 * r : 4 * (r + 1), :],
                        ag_a_out[r, 1].rearrange("(k p) c -> p k c", p=P),
                    )
                nc.vector.tensor_add(
                    d_sb[:], cmp_a_hi[:, :, hA * dh : CWa], cmp_a_lo[:, :, hA * dh : CWa]
                )
                for hh in range(hA):
                    nc.gpsimd.tensor_copy(
                        wh4[:, hh, :, 0:dh], cmp_a_hi[:, :, hh * dh : (hh + 1) * dh]
                    )
                    nc.vector.tensor_copy(
                        wl4[:, hh, :, 0:dh], cmp_a_lo[:, :, hh * dh : (hh + 1) * dh]
                    )
                if hB:
                    cmp_b_hi = layerbuf.tile([P, JT, CWb], dt.bfloat16, tag="cmp_b_hi")
                    cmp_b_lo = layerbuf.tile([P, JT, CWb], dt.bfloat16, tag="cmp_b_lo")
                    for r in range(NCORE):
                        nc.sync.dma_start(
                            cmp_b_hi[:, 4 * r : 4 * (r + 1), :],
                            ag_b_out[r, 0].rearrange("(k p) c -> p k c", p=P),
                        )
                        nc.sync.dma_start(
                            cmp_b_lo[:, 4 * r : 4 * (r + 1), :],
                            ag_b_out[r, 1].rearrange("(k p) c -> p k c", p=P),
                        )
                    for hh in range(hA, h):
                        nc.gpsimd.tensor_copy(
                            wh4[:, hh, :, 0:dh],
                            cmp_b_hi[:, :, (hh - hA) * dh : (hh - hA + 1) * dh],
                        )
                        nc.vector.tensor_copy(
                            wl4[:, hh, :, 0:dh],
                            cmp_b_lo[:, :, (hh - hA) * dh : (hh - hA + 1) * dh],
                        )

                if debug_taps and li == 1:
                    nc.sync.dma_start(dbg_d[:], d_sb[:])
                    dbgw = small.tile([P, 8 * 33], dt.float32, tag="o_sb")
                    nc.vector.tensor_copy(dbgw[:], whrow[:, 7, :])
                    nc.sync.dma_start(dbg_wh[:], dbgw[:])

                # ---- (F) attention in head groups
                G = min(4, hA) if h > 1 else 1
                xnext = xTown_pool.tile([fout, ROWS], dt.float32, tag="xTown")
                for g0 in range(0, h, G):
                    gs = list(range(g0, min(g0 + G, h)))
                    ng = len(gs)
                    att_acc = []
                    for _k in gs:
                        att_t = attps.tile([33, ROWS], dt.float32, tag="att")
                        att_acc.append(att_t)
                    for jt in range(JT):
                        l_jt = work.tile([P, ng * ROWS], dt.float32, tag="l_jt")
                        for k, hh in enumerate(gs):
                            nc.vector._custom_dve(
                                LEAKY_BIAS_ADDMASK,
                                out=l_jt[:, k * ROWS : (k + 1) * ROWS],
                                in0=sreps[hh][:],
                                in1=maskT[:, jt, :],
                                s0=d_sb[:, jt, hh : hh + 1],
                                s1=ALPHA,
                            )
                        p_jt = work.tile([P, ng * ROWS], dt.bfloat16, tag="p_jt")
                        nc.scalar.activation(p_jt[:], l_jt[:], AF.Exp)
                        for k, hh in enumerate(gs):
                            nc.tensor.matmul(
                                att_acc[k][:],
                                whrow[:, jt]
                                .rearrange("p (a b) -> p a b", a=h)[:, hh, :],
                                p_jt[:, k * ROWS : (k + 1) * ROWS],
                                start=(jt == 0),
                                stop=False,
                            )
                            nc.tensor.matmul(
                                att_acc[k][:],
                                whrow_lo[:, jt]
                                .rearrange("p (a b) -> p a b", a=h)[:, hh, :],
                                p_jt[:, k * ROWS : (k + 1) * ROWS],
                                start=False,
                                stop=(jt == JT - 1),
                            )
                    # epilogue per head
                    for k, hh in enumerate(gs):
                        o_sb = small.tile([33, ROWS], dt.float32, tag="o_sb")
                        nc.scalar.copy(o_sb[:], att_acc[k][:])
                        r_sb = small.tile([1, ROWS], dt.float32, tag="vec1")
                        nc.vector.reciprocal(r_sb[:], o_sb[32:33, :])
                        rrep = small.tile([dh, ROWS], dt.float32, tag="rrep")
                        nc.gpsimd.partition_broadcast(rrep[:], r_sb[:])
                        ohead = small.tile([dh, ROWS], dt.float32, tag="ohead")
                        nc.gpsimd.tensor_mul(ohead[:], o_sb[0:dh, :], rrep[:])
                        if elu:
                            # elu(x) = max(x,0) - 1 + exp(min(x,0))
                            mmin = small.tile([dh, ROWS], dt.float32, tag="tmp1")
                            nc.gpsimd.tensor_scalar(
                                mmin[:], ohead[:], 0.0, None, mybir.AluOpType.min
                            )
                            emin = small.tile([dh, ROWS], dt.float32, tag="tmp2")
                            nc.scalar.activation(emin[:], mmin[:], AF.Exp)
                            rmax = small.tile([dh, ROWS], dt.float32, tag="tmp1")
                            nc.gpsimd.tensor_scalar(
                                rmax[:],
                                ohead[:],
                                0.0,
                                -1.0,
                                mybir.AluOpType.max,
                                mybir.AluOpType.add,
                            )
                            nc.gpsimd.tensor_add(ohead[:], rmax[:], emin[:])
                        nc.sync.dma_start(
                            xnext[hh * dh : (hh + 1) * dh, :], ohead[:]
                        )

                if debug_taps:
                    nc.sync.dma_start(dbg_x[li][:], xnext[:])
                if is_last:
                    psum_final = small.tile([fout, 1], dt.float32, tag="vec1")
                    nc.vector.reduce_sum(
                        psum_final[:], xnext[:], axis=mybir.AxisListType.X
                    )
                    nc.sync.dma_start(pool_out[:], psum_final[:])
                else:
                    xTown_cur = [xnext]

    nc.finalize()
    return nc


_NC_CACHE = None
_last_in_maps = None


def kernel(**inputs):
    global _NC_CACHE
    node_features = np.asarray(inputs["node_features"], dtype=np.float32)
    adj = np.ascontiguousarray(np.asarray(inputs["adj_mat"], dtype=np.int32))
    fc_w = np.asarray(inputs["fc_w"], dtype=np.float32)
    fc_b = np.asarray(inputs["fc_b"], dtype=np.float32)

    x0T = node_features.T  # [256, N]

    wext = {}
    ws = {}
    for li, (fin, fout, h, concat, _elu) in enumerate(CFG, start=1):
        dh = fout // h if concat else fout
        W = np.asarray(inputs[f"W{li}"], dtype=np.float32)  # [h, fin, dh]
        a_src = np.asarray(inputs[f"a_src{li}"], dtype=np.float32)  # [h, dh]
        a_dst = np.asarray(inputs[f"a_dst{li}"], dtype=np.float32)
        wcat = W.transpose(1, 0, 2).reshape(fin, h * dh)
        wd = np.einsum("hfd,hd->fh", W, a_dst).astype(np.float32)
        wsrc = np.einsum("hfd,hd->fh", W, a_src).astype(np.float32)
        import os as _os2
        hA = (h // 2 if h > 1 else 1) if not _os2.environ.get("NOSPLIT") else h
        wext[li] = np.ascontiguousarray(
            np.concatenate([wcat[:, : hA * dh], wd, wcat[:, hA * dh :]], axis=1)
        )
        ws[li] = np.ascontiguousarray(wsrc)

    in_maps = []
    for c in range(NCORE):
        m = {
            "adjrows": np.ascontiguousarray(adj[c * ROWS : (c + 1) * ROWS, :]),
            "x0T_own": np.ascontiguousarray(x0T[:, c * ROWS : (c + 1) * ROWS]),
        }
        for li in range(1, 6):
            m[f"wext{li}"] = wext[li]
            m[f"ws{li}"] = ws[li]
        in_maps.append(m)

    if _NC_CACHE is None:
        _NC_CACHE = build_kernel()
    nc = _NC_CACHE
    global _last_in_maps
    _last_in_maps = in_maps

    res = run_bass_kernel_spmd(nc, in_maps, list(range(NCORE)))
    total = np.zeros((8,), dtype=np.float32)
    for c in range(NCORE):
        total += res.results[c]["pool_part"][:, 0]
    pooled = total / np.float32(N)
    out = pooled @ fc_w + fc_b
    return out.astype(np.float32)



# revision 3
# speedup vs baseline: 1.6006x; 1.6006x over previous
"""GAT (5-layer, dense-adjacency) Trainium2 kernel, sharded across 8 NeuronCores.

Sharding: query-node rows split 512/core. Each core holds its transposed
additive attention mask [4096(j), 512(i)] resident in SBUF (bf16), computes
its own row-block of Wh per layer, AllGathers the full Wh (+d column) in
bf16, then computes its row-block of attention via a fused DVE op
(leaky(s+d)+addmask) + ACT exp + a single bf16 matmul per (head, j-tile)
with a ones-column for softmax denominators.
"""

import numpy as np

import concourse.bacc as bacc
import concourse.mybir as mybir
import concourse.tile as tile
from concourse.bass_utils import run_bass_kernel_spmd

import concourse.dve_ops as dve_ops
from concourse.dve_spec import Spec, Src0, Src1, C0, C1, maxx, lower
from concourse.dve_spec import _has_src1 as _spec_has_src1
from concourse.dve_uop import DveOpSpec

try:
    import ml_dtypes

    _BF16 = ml_dtypes.bfloat16
except ImportError:  # pragma: no cover
    _BF16 = np.float32

dt = mybir.dt
AF = mybir.ActivationFunctionType

# ---------------------------------------------------------------- constants
N = 4096
NCORE = 8
ROWS = N // NCORE  # 512 query rows per core
P = 128
JT = N // P  # 32 j-tiles
NEG = -30000.0  # additive mask for non-edges; exp(x-30000) == 0
ALPHA = 0.1
# (fin, fout, heads, concat, elu_after)
CFG = [
    (256, 128, 8, True, True),
    (128, 64, 8, True, True),
    (64, 32, 4, True, True),
    (32, 16, 1, True, False),
    (16, 8, 1, False, False),
]

# ---------------------------------------------------------------- custom op
LEAKY_BIAS_ADDMASK = dve_ops.DveOp(
    "LEAKY_BIAS_ADDMASK",
    Spec(
        body=maxx(Src0 + C0, (Src0 + C0) * C1) + Src1,
        reference=lambda in0, in1, s0, s1, imm2: (
            np.maximum(in0 + s0, (in0 + s0) * s1) + in1
        ).astype(np.float32),
    ),
    subdim=False,
    uops_sha={},
)


def _register_custom_op(op):
    if op.name in dve_ops._SUB_OPCODE_FOR_NAME:
        return
    idx = dve_ops._CUSTOM_DVE_ROW_BASE + len(dve_ops.OPS)
    assert idx < 0x20
    dve_ops.OPS.append(op)
    dve_ops.CUSTOM_DVE_SPECS[op.name] = op.spec
    dve_ops._SUB_OPCODE_FOR_NAME[op.name] = idx
    shas = {}
    for ver in ("v3", "v4"):
        try:
            s = DveOpSpec(
                name=op.name,
                opcode=idx,
                uops=lower(op.spec, ver=ver),
                rd1_en=_spec_has_src1(op.spec),
            )
            shas[ver] = s.sha(ver)
        except Exception:
            pass
    object.__setattr__(op, "uops_sha", shas)


_register_custom_op(LEAKY_BIAS_ADDMASK)


# ---------------------------------------------------------------- builder
def build_kernel():
    import os as _os

    debug_taps = bool(_os.environ.get("DEBUG_TAPS"))
    nc = bacc.Bacc("TRN2", target_bir_lowering=False, debug=False)

    adjrows = nc.dram_tensor("adjrows", [ROWS, N], dt.int32, kind="ExternalInput")
    x0T_own = nc.dram_tensor("x0T_own", [256, ROWS], dt.float32, kind="ExternalInput")
    wext_dram = {}
    ws_dram = {}
    for li, (fin, fout, h, concat, _elu) in enumerate(CFG, start=1):
        dh = fout // h if concat else fout
        wext_dram[li] = nc.dram_tensor(
            f"wext{li}", [fin, h * dh + h], dt.float32, kind="ExternalInput"
        )
        ws_dram[li] = nc.dram_tensor(f"ws{li}", [fin, h], dt.float32, kind="ExternalInput")

    pool_out = nc.dram_tensor("pool_part", [8, 1], dt.float32, kind="ExternalOutput")
    if debug_taps:
        dbg_x = {}
        for _li, (_f, _fo, _h, _c, _e) in enumerate(CFG, start=1):
            dbg_x[_li] = nc.dram_tensor(
                f"dbg_x{_li}", [_fo, ROWS], dt.float32, kind="ExternalOutput"
            )

    ident_np = np.eye(P, dtype=_BF16)
    ident_dram = nc.inline_tensor(ident_np, name="ident128")

    with tile.TileContext(nc) as tc:
        with (
            tc.tile_pool(name="persist", bufs=1) as persist,
            tc.tile_pool(name="dram", bufs=1, space="DRAM") as drampool,
            tc.tile_pool(name="xTown", bufs=3) as xTown_pool,
            tc.tile_pool(name="layerbuf", bufs=1) as layerbuf,
            tc.tile_pool(name="ownp", bufs=2) as ownp,
            tc.tile_pool(name="srep", bufs=1) as srep_pool,
            tc.tile_pool(name="work", bufs=2) as work,
            tc.tile_pool(name="small", bufs=2) as small,
            tc.tile_pool(name="whps", bufs=1, space="PSUM") as whps,
            tc.tile_pool(name="sps", bufs=2, space="PSUM") as sps,
            tc.tile_pool(name="trps", bufs=1, space="PSUM") as trps,
            tc.tile_pool(name="attps", bufs=4, space="PSUM") as attps,
        ):
            # ---------------- persistent tiles
            maskT = persist.tile([P, JT, ROWS], dt.bfloat16, tag="maskT")
            ident_sb = persist.tile([P, P], dt.bfloat16, tag="ident")
            nc.sync.dma_start(ident_sb[:], ident_dram[:])
            ones_row = persist.tile([1, P], dt.float32, tag="ones_row")
            nc.vector.memset(ones_row[:], 1.0)

            wext_sb = {}
            ws_sb = {}
            for li, (fin, fout, h, concat, _elu) in enumerate(CFG, start=1):
                dh = fout // h if concat else fout
                nft = (fin + P - 1) // P
                wext_sb[li] = []
                ws_sb[li] = []
                for ft in range(nft):
                    fr = min(P, fin - ft * P)
                    wt = persist.tile([fr, h * dh + h], dt.float32, tag=f"wext{li}_{ft}")
                    nc.sync.dma_start(wt[:], wext_dram[li][ft * P : ft * P + fr, :])
                    wext_sb[li].append(wt)
                    st = persist.tile([fr, h], dt.float32, tag=f"ws{li}_{ft}")
                    nc.sync.dma_start(st[:], ws_dram[li][ft * P : ft * P + fr, :])
                    ws_sb[li].append(st)

            # ---------------- L1 own activations from input
            xTown_cur = []
            for ft in range(2):
                to = xTown_pool.tile([P, ROWS], dt.float32, tag="xTown")
                nc.sync.dma_start(to[:], x0T_own[ft * P : (ft + 1) * P, :])
                xTown_cur.append(to)

            mask_built = False

            def build_mask():
                # transpose adj rows -> additive maskT (bf16).
                CH = 1024
                for c0 in range(0, N, CH):
                    for ib in range(ROWS // P):
                        stage_i = work.tile([P, CH], dt.int32, tag="stage_i")
                        nc.sync.dma_start(
                            stage_i[:], adjrows[ib * P : (ib + 1) * P, c0 : c0 + CH]
                        )
                        stage_b = work.tile([P, CH], dt.bfloat16, tag="stage_b")
                        nc.gpsimd.tensor_copy(stage_b[:], stage_i[:])
                        for k in range(CH // P):
                            jt = (c0 + k * P) // P
                            tps = trps.tile([P, P], dt.bfloat16, tag="tps")
                            nc.tensor.transpose(
                                tps[:], stage_b[:, k * P : (k + 1) * P], ident_sb[:]
                            )
                            # adj -> additive mask: adj*(-NEG) + NEG
                            nc.vector.tensor_scalar(
                                maskT[:, jt, ib * P : (ib + 1) * P],
                                tps[:],
                                -NEG,
                                NEG,
                                mybir.AluOpType.mult,
                                mybir.AluOpType.add,
                            )

            for li, (fin, fout, h, concat, elu) in enumerate(CFG, start=1):
                dh = fout // h if concat else fout
                hdh = h * dh
                CW = hdh + h  # compact row width: Wh values + d column
                nft = (fin + P - 1) // P
                is_last = li == len(CFG)

                # ---- (A) own-block Wh (+d) for the 4 own j-chunks -> bf16
                own_sb = ownp.tile([P, 4, CW], dt.bfloat16, tag="own_sb")
                for k in range(4):
                    pw = whps.tile([P, CW], dt.float32, tag="pw")
                    for ft in range(nft):
                        fr = min(P, fin - ft * P)
                        nc.tensor.matmul(
                            pw[:],
                            xTown_cur[ft][0:fr, k * P : (k + 1) * P],
                            wext_sb[li][ft][:],
                            start=(ft == 0),
                            stop=(ft == nft - 1),
                        )
                    nc.scalar.copy(own_sb[:, k, :], pw[:])

                # ---- (C) AllGather the compact Wh+d rows
                ag_in = drampool.tile([4 * P, CW], dt.bfloat16, tag=f"agin{li}")
                ag_out = drampool.tile(
                    [NCORE, 4 * P, CW],
                    dt.bfloat16,
                    tag=f"agout{li}",
                    addr_space="Shared",
                )
                nc.sync.dma_start(
                    ag_in.rearrange("(k p) c -> p k c", p=P), own_sb[:]
                )
                nc.gpsimd.collective_compute(
                    "AllGather",
                    mybir.AluOpType.bypass,
                    replica_groups=[list(range(NCORE))],
                    ins=[ag_in.opt()],
                    outs=[ag_out.opt()],
                )

                # ---- (B) srep per head (from own activations; AG-independent)
                sreps = []
                for hh in range(h):
                    ps_row = sps.tile([1, ROWS], dt.float32, tag="ps_row")
                    for ft in range(nft):
                        fr = min(P, fin - ft * P)
                        nc.tensor.matmul(
                            ps_row[:],
                            ws_sb[li][ft][:, hh : hh + 1],
                            xTown_cur[ft][0:fr, :],
                            start=(ft == 0),
                            stop=(ft == nft - 1),
                        )
                    s_row = small.tile([1, ROWS], dt.float32, tag="vec1")
                    nc.scalar.copy(s_row[:], ps_row[:])
                    srt = srep_pool.tile([P, ROWS], dt.float32, tag=f"srep{hh}")
                    nc.gpsimd.partition_broadcast(srt[:], s_row[:])
                    sreps.append(srt)

                # L1: mask build overlaps the first AllGather
                if not mask_built:
                    build_mask()
                    mask_built = True

                # ---- (D) unpack AG result into padded matmul layout
                whrow = layerbuf.tile([P, JT, h * 33], dt.bfloat16, tag="whrow")
                wh4 = whrow.rearrange("p j (a b) -> p a j b", a=h)
                for hh in range(h):
                    nc.gpsimd.memset(wh4[:, hh, :, dh:32], 0.0)
                    nc.gpsimd.memset(wh4[:, hh, :, 32:33], 1.0)
                ag_view = ag_out.rearrange("r (k p) c -> p (r k) c", p=P)
                for hh in range(h):
                    nc.sync.dma_start(
                        wh4[:, hh, :, 0:dh],
                        ag_view[:, :, hh * dh : (hh + 1) * dh],
                    )
                d_bf = layerbuf.tile([P, JT, h], dt.bfloat16, tag="d_bf")
                nc.sync.dma_start(d_bf[:], ag_view[:, :, hdh:CW])
                d_sb = layerbuf.tile([P, JT, h], dt.float32, tag="d_sb")
                nc.gpsimd.tensor_copy(d_sb[:], d_bf[:])

                # ---- (F) attention in head groups
                G = min(4, h)
                xnext = xTown_pool.tile([fout, ROWS], dt.float32, tag="xTown")
                for g0 in range(0, h, G):
                    gs = list(range(g0, min(g0 + G, h)))
                    ng = len(gs)
                    att_acc = []
                    for _k in gs:
                        att_t = attps.tile([33, ROWS], dt.float32, tag="att")
                        att_acc.append(att_t)
                    for jt in range(JT):
                        l_jt = work.tile([P, ng * ROWS], dt.float32, tag="l_jt")
                        for k, hh in enumerate(gs):
                            nc.vector._custom_dve(
                                LEAKY_BIAS_ADDMASK,
                                out=l_jt[:, k * ROWS : (k + 1) * ROWS],
                                in0=sreps[hh][:],
                                in1=maskT[:, jt, :],
                                s0=d_sb[:, jt, hh : hh + 1],
                                s1=ALPHA,
                            )
                        p_jt = work.tile([P, ng * ROWS], dt.bfloat16, tag="p_jt")
                        nc.scalar.activation(p_jt[:], l_jt[:], AF.Exp)
                        for k, hh in enumerate(gs):
                            nc.tensor.matmul(
                                att_acc[k][:],
                                whrow[:, jt]
                                .rearrange("p (a b) -> p a b", a=h)[:, hh, :],
                                p_jt[:, k * ROWS : (k + 1) * ROWS],
                                start=(jt == 0),
                                stop=(jt == JT - 1),
                            )
                    # epilogue per head
                    for k, hh in enumerate(gs):
                        o_sb = small.tile([33, ROWS], dt.float32, tag="o_sb")
                        nc.scalar.copy(o_sb[:], att_acc[k][:])
                        r_sb = small.tile([1, ROWS], dt.float32, tag="vec1")
                        nc.vector.reciprocal(r_sb[:], o_sb[32:33, :])
                        rrep = small.tile([dh, ROWS], dt.float32, tag="rrep")
                        nc.gpsimd.partition_broadcast(rrep[:], r_sb[:])
                        ohead = small.tile([dh, ROWS], dt.float32, tag="ohead")
                        nc.gpsimd.tensor_mul(ohead[:], o_sb[0:dh, :], rrep[:])
                        if elu:
                            # elu(x) = max(x,0) - 1 + exp(min(x,0))
                            mmin = small.tile([dh, ROWS], dt.float32, tag="tmp1")
                            nc.gpsimd.tensor_scalar(
                                mmin[:], ohead[:], 0.0, None, mybir.AluOpType.min
                            )
                            emin = small.tile([dh, ROWS], dt.float32, tag="tmp2")
                            nc.scalar.activation(emin[:], mmin[:], AF.Exp)
                            rmax = small.tile([dh, ROWS], dt.float32, tag="tmp1")
                            nc.gpsimd.tensor_scalar(
                                rmax[:],
                                ohead[:],
                                0.0,
                                -1.0,
                                mybir.AluOpType.max,
                                mybir.AluOpType.add,
                            )
                            nc.gpsimd.tensor_add(ohead[:], rmax[:], emin[:])
                        nc.sync.dma_start(
                            xnext[hh * dh : (hh + 1) * dh, :], ohead[:]
                        )

                if debug_taps:
                    nc.sync.dma_start(dbg_x[li][:], xnext[:])
                if is_last:
                    psum_final = small.tile([fout, 1], dt.float32, tag="vec1f")
                    nc.vector.reduce_sum(
                        psum_final[:], xnext[:], axis=mybir.AxisListType.X
                    )
                    nc.sync.dma_start(pool_out[:], psum_final[:])
                else:
                    xTown_cur = [xnext]

    nc.finalize()
    return nc


_NC_CACHE = None
_last_in_maps = None


def kernel(**inputs):
    global _NC_CACHE
    node_features = np.asarray(inputs["node_features"], dtype=np.float32)
    adj = np.ascontiguousarray(np.asarray(inputs["adj_mat"], dtype=np.int32))
    fc_w = np.asarray(inputs["fc_w"], dtype=np.float32)
    fc_b = np.asarray(inputs["fc_b"], dtype=np.float32)

    x0T = node_features.T  # [256, N]

    wext = {}
    ws = {}
    for li, (fin, fout, h, concat, _elu) in enumerate(CFG, start=1):
        dh = fout // h if concat else fout
        W = np.asarray(inputs[f"W{li}"], dtype=np.float32)  # [h, fin, dh]
        a_src = np.asarray(inputs[f"a_src{li}"], dtype=np.float32)  # [h, dh]
        a_dst = np.asarray(inputs[f"a_dst{li}"], dtype=np.float32)
        wcat = W.transpose(1, 0, 2).reshape(fin, h * dh)
        wd = np.einsum("hfd,hd->fh", W, a_dst).astype(np.float32)
        wsrc = np.einsum("hfd,hd->fh", W, a_src).astype(np.float32)
        wext[li] = np.ascontiguousarray(np.concatenate([wcat, wd], axis=1))
        ws[li] = np.ascontiguousarray(wsrc)

    in_maps = []
    for c in range(NCORE):
        m = {
            "adjrows": np.ascontiguousarray(adj[c * ROWS : (c + 1) * ROWS, :]),
            "x0T_own": np.ascontiguousarray(x0T[:, c * ROWS : (c + 1) * ROWS]),
        }
        for li in range(1, 6):
            m[f"wext{li}"] = wext[li]
            m[f"ws{li}"] = ws[li]
        in_maps.append(m)

    if _NC_CACHE is None:
        _NC_CACHE = build_kernel()
    nc = _NC_CACHE
    global _last_in_maps
    _last_in_maps = in_maps

    res = run_bass_kernel_spmd(nc, in_maps, list(range(NCORE)))
    total = np.zeros((8,), dtype=np.float32)
    for c in range(NCORE):
        total += res.results[c]["pool_part"][:, 0]
    pooled = total / np.float32(N)
    out = pooled @ fc_w + fc_b
    return out.astype(np.float32)


# revision 11
# speedup vs baseline: 1.6050x; 1.0028x over previous
"""GAT (5-layer, dense-adjacency) Trainium2 kernel, sharded across 8 NeuronCores.

Sharding: query-node rows split 512/core. Per layer each core computes its
own row-block of Wh, AllGathers d-score partials (f32, launched per
head-group as soon as that group's epilogue lands so the gather overlaps the
next group's compute) plus the Wh values (bf16, overlapped with the score
pipeline), then computes its row-block of attention: fused DVE op
(leaky(s+d)+addmask) + ACT exp + one bf16 matmul per (head, j-tile) with a
ones-column for softmax denominators.
"""

import numpy as np

import concourse.bacc as bacc
import concourse.mybir as mybir
import concourse.tile as tile
from concourse.bass_utils import run_bass_kernel_spmd

import concourse.dve_ops as dve_ops
from concourse.dve_spec import Spec, Src0, Src1, C0, C1, maxx, lower
from concourse.dve_spec import _has_src1 as _spec_has_src1
from concourse.dve_uop import DveOpSpec

try:
    import ml_dtypes

    _BF16 = ml_dtypes.bfloat16
except ImportError:  # pragma: no cover
    _BF16 = np.float32

dt = mybir.dt
AF = mybir.ActivationFunctionType

# ---------------------------------------------------------------- constants
N = 4096
NCORE = 8
ROWS = N // NCORE  # 512 query rows per core
P = 128
JT = N // P  # 32 j-tiles
NEG = -30000.0  # additive mask for non-edges; exp(x-30000) == 0
ALPHA = 0.1
# (fin, fout, heads, concat, elu_after, group_size)
CFG = [
    (256, 128, 8, True, True, 4),
    (128, 64, 8, True, True, 4),
    (64, 32, 4, True, True, 2),
    (32, 16, 1, True, False, 1),
    (16, 8, 1, False, False, 1),
]

# ---------------------------------------------------------------- custom op
LEAKY_BIAS_ADDMASK = dve_ops.DveOp(
    "LEAKY_BIAS_ADDMASK",
    Spec(
        body=maxx(Src0 + C0, (Src0 + C0) * C1) + Src1,
        reference=lambda in0, in1, s0, s1, imm2: (
            np.maximum(in0 + s0, (in0 + s0) * s1) + in1
        ).astype(np.float32),
    ),
    subdim=False,
    uops_sha={},
)


def _register_custom_op(op):
    if op.name in dve_ops._SUB_OPCODE_FOR_NAME:
        return
    idx = dve_ops._CUSTOM_DVE_ROW_BASE + len(dve_ops.OPS)
    assert idx < 0x20
    dve_ops.OPS.append(op)
    dve_ops.CUSTOM_DVE_SPECS[op.name] = op.spec
    dve_ops._SUB_OPCODE_FOR_NAME[op.name] = idx
    shas = {}
    for ver in ("v3", "v4"):
        try:
            s = DveOpSpec(
                name=op.name,
                opcode=idx,
                uops=lower(op.spec, ver=ver),
                rd1_en=_spec_has_src1(op.spec),
            )
            shas[ver] = s.sha(ver)
        except Exception:
            pass
    object.__setattr__(op, "uops_sha", shas)


_register_custom_op(LEAKY_BIAS_ADDMASK)


def _groups(h, g):
    return [list(range(g0, min(g0 + g, h))) for g0 in range(0, h, g)]


# ---------------------------------------------------------------- builder
def build_kernel():
    import os as _os

    debug_taps = bool(_os.environ.get("DEBUG_TAPS"))
    nc = bacc.Bacc("TRN2", target_bir_lowering=False, debug=False)

    adjrows = nc.dram_tensor("adjrows", [ROWS, N], dt.int32, kind="ExternalInput")
    x0T_own = nc.dram_tensor("x0T_own", [256, ROWS], dt.float32, kind="ExternalInput")
    wext_dram = {}
    wd_dram = {}
    ws_dram = {}
    for li, (fin, fout, h, concat, _elu, _g) in enumerate(CFG, start=1):
        dh = fout // h if concat else fout
        wext_dram[li] = nc.dram_tensor(
            f"wext{li}", [fin, h * dh], dt.float32, kind="ExternalInput"
        )
        wd_dram[li] = nc.dram_tensor(f"wd{li}", [fin, h], dt.float32, kind="ExternalInput")
        ws_dram[li] = nc.dram_tensor(f"ws{li}", [fin, h], dt.float32, kind="ExternalInput")

    pool_out = nc.dram_tensor("pool_part", [8, 1], dt.float32, kind="ExternalOutput")
    if debug_taps:
        dbg_x = {}
        for _li, (_f, _fo, _h, _c, _e, _g) in enumerate(CFG, start=1):
            dbg_x[_li] = nc.dram_tensor(
                f"dbg_x{_li}", [_fo, ROWS], dt.float32, kind="ExternalOutput"
            )

    ident_np = np.eye(P, dtype=_BF16)
    ident_dram = nc.inline_tensor(ident_np, name="ident128")

    with tile.TileContext(nc) as tc:
        with (
            tc.tile_pool(name="persist", bufs=1) as persist,
            tc.tile_pool(name="dram", bufs=1, space="DRAM") as drampool,
            tc.tile_pool(name="xTown", bufs=3) as xTown_pool,
            tc.tile_pool(name="layerbuf", bufs=2) as layerbuf,
            tc.tile_pool(name="ownp", bufs=2) as ownp,
            tc.tile_pool(name="srep", bufs=1) as srep_pool,
            tc.tile_pool(name="work", bufs=2) as work,
            tc.tile_pool(name="pjt", bufs=3) as pjt_pool,
            tc.tile_pool(name="small", bufs=2) as small,
            tc.tile_pool(name="whps", bufs=1, space="PSUM") as whps,
            tc.tile_pool(name="sps", bufs=1, space="PSUM") as sps,
            tc.tile_pool(name="trps", bufs=1, space="PSUM") as trps,
            tc.tile_pool(name="attps", bufs=4, space="PSUM") as attps,
        ):
            # ---------------- persistent tiles
            maskT = persist.tile([P, JT, ROWS], dt.bfloat16, tag="maskT")
            ident_sb = persist.tile([P, P], dt.bfloat16, tag="ident")
            nc.sync.dma_start(ident_sb[:], ident_dram[:])
            ones_row = persist.tile([1, P], dt.float32, tag="ones_row")
            nc.vector.memset(ones_row[:], 1.0)

            wext_sb = {}
            wd_sb = {}
            ws_sb = {}
            for li, (fin, fout, h, concat, _elu, _g) in enumerate(CFG, start=1):
                dh = fout // h if concat else fout
                nft = (fin + P - 1) // P
                wext_sb[li] = []
                wd_sb[li] = []
                ws_sb[li] = []
                for ft in range(nft):
                    fr = min(P, fin - ft * P)
                    wt = persist.tile([fr, h * dh], dt.float32, tag=f"wext{li}_{ft}")
                    nc.sync.dma_start(wt[:], wext_dram[li][ft * P : ft * P + fr, :])
                    wext_sb[li].append(wt)
                    dtl = persist.tile([fr, h], dt.float32, tag=f"wd{li}_{ft}")
                    nc.sync.dma_start(dtl[:], wd_dram[li][ft * P : ft * P + fr, :])
                    wd_sb[li].append(dtl)
                    st = persist.tile([fr, h], dt.float32, tag=f"ws{li}_{ft}")
                    nc.sync.dma_start(st[:], ws_dram[li][ft * P : ft * P + fr, :])
                    ws_sb[li].append(st)

            # ---------------- L1 own activations from input
            xTown_cur = []
            for ft in range(2):
                to = xTown_pool.tile([P, ROWS], dt.float32, tag="xTown")
                nc.sync.dma_start(to[:], x0T_own[ft * P : (ft + 1) * P, :])
                xTown_cur.append(to)

            def build_mask():
                # transpose adj rows -> additive maskT (bf16).
                CH = 1024
                for c0 in range(0, N, CH):
                    for ib in range(ROWS // P):
                        stage_i = work.tile([P, CH], dt.int32, tag="stage_i")
                        nc.sync.dma_start(
                            stage_i[:], adjrows[ib * P : (ib + 1) * P, c0 : c0 + CH]
                        )
                        stage_b = work.tile([P, CH], dt.bfloat16, tag="stage_b")
                        nc.gpsimd.tensor_copy(stage_b[:], stage_i[:])
                        for k in range(CH // P):
                            jt = (c0 + k * P) // P
                            tps = trps.tile([P, P], dt.bfloat16, tag="tps")
                            nc.tensor.transpose(
                                tps[:], stage_b[:, k * P : (k + 1) * P], ident_sb[:]
                            )
                            # adj -> additive mask: adj*(-NEG) + NEG
                            nc.vector.tensor_scalar(
                                maskT[:, jt, ib * P : (ib + 1) * P],
                                tps[:],
                                -NEG,
                                NEG,
                                mybir.AluOpType.mult,
                                mybir.AluOpType.add,
                            )

            def emit_partial_d(li_next, fg0, fg1, x_tiles, tag):
                """Own-row partial d for layer li_next from feature rows
                [fg0:fg1) of x (list of [<=128, ROWS] tiles). Returns the AG
                output DRAM tile [NCORE, 4P, h_next] f32."""
                h_next = CFG[li_next - 1][2]
                agd_in = drampool.tile([4 * P, h_next], dt.float32, tag=f"agdin{tag}")
                agd_out = drampool.tile(
                    [NCORE, 4 * P, h_next],
                    dt.float32,
                    tag=f"agdout{tag}",
                    addr_space="Shared",
                )
                pd_sb = small.tile([P, 4, h_next], dt.float32, tag="pd_sb")
                for k in range(4):
                    pd = whps.tile([P, h_next], dt.float32, tag="pw")
                    f = fg0
                    first = True
                    while f < fg1:
                        ft = f // P
                        fe = min(fg1, (ft + 1) * P)
                        nc.tensor.matmul(
                            pd[:],
                            x_tiles[ft][f - ft * P : fe - ft * P, k * P : (k + 1) * P],
                            wd_sb[li_next][ft][f - ft * P : fe - ft * P, :],
                            start=first,
                            stop=(fe == fg1),
                        )
                        first = False
                        f = fe
                    nc.scalar.copy(pd_sb[:, k, :], pd[:])
                nc.sync.dma_start(
                    agd_in.rearrange("(k p) c -> p k c", p=P), pd_sb[:]
                )
                nc.gpsimd.collective_compute(
                    "AllGather",
                    mybir.AluOpType.bypass,
                    replica_groups=[list(range(NCORE))],
                    ins=[agd_in.opt()],
                    outs=[agd_out.opt()],
                )
                return agd_out

            # L1 d: from input features (both feature tiles at once)
            d_pending = [emit_partial_d(1, 0, 256, xTown_cur, "l1")]

            for li, (fin, fout, h, concat, elu, G) in enumerate(CFG, start=1):
                dh = fout // h if concat else fout
                hdh = h * dh
                nft = (fin + P - 1) // P
                is_last = li == len(CFG)
                groups = _groups(h, G)

                # ---- assemble d from pending partial-AGs
                d_parts = []
                for pi, agd_out in enumerate(d_pending):
                    dp = layerbuf.tile([P, JT, h], dt.float32, tag=f"dpart{pi}")
                    nc.sync.dma_start(
                        dp[:], agd_out.rearrange("r (k p) c -> p (r k) c", p=P)
                    )
                    d_parts.append(dp)
                if len(d_parts) == 1:
                    d_sb = d_parts[0]
                else:
                    d_sb = layerbuf.tile([P, JT, h], dt.float32, tag="d_sb")
                    nc.gpsimd.tensor_add(d_sb[:], d_parts[0][:], d_parts[1][:])

                # ---- sreps (AG-independent)
                sreps = []
                for hh in range(h):
                    ps_row = sps.tile([1, ROWS], dt.float32, tag="ps_row")
                    for ft in range(nft):
                        fr = min(P, fin - ft * P)
                        nc.tensor.matmul(
                            ps_row[:],
                            ws_sb[li][ft][:, hh : hh + 1],
                            xTown_cur[ft][0:fr, :],
                            start=(ft == 0),
                            stop=(ft == nft - 1),
                        )
                    s_row = small.tile([1, ROWS], dt.float32, tag="vec1")
                    nc.scalar.copy(s_row[:], ps_row[:])
                    srt = srep_pool.tile([P, ROWS], dt.float32, tag=f"srep{hh}")
                    nc.gpsimd.partition_broadcast(srt[:], s_row[:])
                    sreps.append(srt)

                if li == 1:
                    build_mask()

                # ---- own-block Wh values (+ones col per head) -> AllGather
                # layout per row: h blocks of [dh values | 1.0], so the gathered
                # tensor is directly the matmul stationary (ones col gives the
                # softmax denominator).
                dh1 = dh + 1
                cw = h * dh1
                own_sb = ownp.tile([P, 4, cw], dt.bfloat16, tag="own_sb")
                own4 = own_sb.rearrange("p k (a b) -> p k a b", a=h)
                nc.gpsimd.memset(own4[:, :, :, dh : dh + 1], 1.0)
                for k in range(4):
                    pw = whps.tile([P, hdh], dt.float32, tag="pw")
                    for ft in range(nft):
                        fr = min(P, fin - ft * P)
                        nc.tensor.matmul(
                            pw[:],
                            xTown_cur[ft][0:fr, k * P : (k + 1) * P],
                            wext_sb[li][ft][:],
                            start=(ft == 0),
                            stop=(ft == nft - 1),
                        )
                    nc.scalar.copy(
                        own4[:, k, :, 0:dh],
                        pw[:].rearrange("p (a b) -> p a b", a=h),
                    )
                ag_in = drampool.tile([4 * P, cw], dt.bfloat16, tag=f"agin{li}")
                ag_out = drampool.tile(
                    [NCORE, 4 * P, cw],
                    dt.bfloat16,
                    tag=f"agout{li}",
                    addr_space="Shared",
                )
                nc.sync.dma_start(ag_in.rearrange("(k p) c -> p k c", p=P), own_sb[:])
                nc.gpsimd.collective_compute(
                    "AllGather",
                    mybir.AluOpType.bypass,
                    replica_groups=[list(range(NCORE))],
                    ins=[ag_in.opt()],
                    outs=[ag_out.opt()],
                )

                # ---- unpack Wh (one DMA)
                whrow = layerbuf.tile([P, JT, cw], dt.bfloat16, tag="whrow")
                nc.sync.dma_start(
                    whrow[:], ag_out.rearrange("r (k p) c -> p (r k) c", p=P)
                )

                # ---- attention per head group
                xnext = xTown_pool.tile([fout, ROWS], dt.float32, tag="xTown")
                n_groups = len(groups)
                for gi, gs in enumerate(groups):
                    ng = len(gs)
                    last_group = gi == n_groups - 1
                    att_acc = {}
                    for hh in gs:
                        att_acc[hh] = attps.tile(
                            [dh1, ROWS], dt.float32, tag="att", name=f"att{hh}"
                        )
                    for jt in range(JT):
                        l_jt = work.tile([P, ng * ROWS], dt.float32, tag="l_jt")
                        for k, hh in enumerate(gs):
                            nc.vector._custom_dve(
                                LEAKY_BIAS_ADDMASK,
                                out=l_jt[:, k * ROWS : (k + 1) * ROWS],
                                in0=sreps[hh][:],
                                in1=maskT[:, jt, :],
                                s0=d_sb[:, jt, hh : hh + 1],
                                s1=ALPHA,
                            )
                        p_jt = pjt_pool.tile([P, ng * ROWS], dt.bfloat16, tag="p_jt")
                        nc.scalar.activation(p_jt[:], l_jt[:], AF.Exp)
                        for k, hh in enumerate(gs):
                            nc.tensor.matmul(
                                att_acc[hh][:],
                                whrow[:, jt, hh * dh1 : (hh + 1) * dh1],
                                p_jt[:, k * ROWS : (k + 1) * ROWS],
                                start=(jt == 0),
                                stop=(jt == JT - 1),
                            )
                    # epilogue per head; last group splits across engines to
                    # shorten the inter-layer critical path
                    for k, hh in enumerate(gs):
                        dve_path = last_group and (k % 2 == 0)
                        o_sb = small.tile([dh1, ROWS], dt.float32, tag="o_sb")
                        nc.scalar.copy(o_sb[:], att_acc[hh][:])
                        r_sb = small.tile([1, ROWS], dt.float32, tag="vec1")
                        nc.vector.reciprocal(r_sb[:], o_sb[dh : dh + 1, :])
                        ohead = small.tile([dh, ROWS], dt.float32, tag="ohead")
                        if dve_path:
                            rps = trps.tile([dh, ROWS], dt.float32, tag="rps")
                            nc.tensor.matmul(
                                rps[:], ones_row[0:1, 0:dh], r_sb[:],
                                start=True, stop=True,
                            )
                            nc.vector.tensor_mul(ohead[:], o_sb[0:dh, :], rps[:])
                        else:
                            rrep = small.tile([dh, ROWS], dt.float32, tag="rrep")
                            nc.gpsimd.partition_broadcast(rrep[:], r_sb[:])
                            nc.gpsimd.tensor_mul(ohead[:], o_sb[0:dh, :], rrep[:])
                        if elu:
                            # elu(x) = max(x,0) - 1 + exp(min(x,0))
                            mmin = small.tile([dh, ROWS], dt.float32, tag="tmp1")
                            emin = small.tile([dh, ROWS], dt.float32, tag="tmp2")
                            rmax = small.tile([dh, ROWS], dt.float32, tag="tmp3")
                            if dve_path:
                                nc.vector.tensor_scalar(
                                    mmin[:], ohead[:], 0.0, None, mybir.AluOpType.min
                                )
                                nc.scalar.activation(emin[:], mmin[:], AF.Exp)
                                nc.vector.tensor_scalar(
                                    rmax[:], ohead[:], 0.0, -1.0,
                                    mybir.AluOpType.max, mybir.AluOpType.add,
                                )
                                nc.vector.tensor_add(ohead[:], rmax[:], emin[:])
                            else:
                                nc.gpsimd.tensor_scalar(
                                    mmin[:], ohead[:], 0.0, None, mybir.AluOpType.min
                                )
                                nc.scalar.activation(emin[:], mmin[:], AF.Exp)
                                nc.gpsimd.tensor_scalar(
                                    rmax[:], ohead[:], 0.0, -1.0,
                                    mybir.AluOpType.max, mybir.AluOpType.add,
                                )
                                nc.gpsimd.tensor_add(ohead[:], rmax[:], emin[:])
                        nc.sync.dma_start(xnext[hh * dh : (hh + 1) * dh, :], ohead[:])
                    # launch next layer's partial-d gather for this group
                    # (matmul base partitions must be 0/32/64-aligned)
                    if not is_last:
                        split_ok = all(g[0] * dh % 32 == 0 for g in groups)
                        if split_ok:
                            fg0 = gs[0] * dh
                            fg1 = (gs[-1] + 1) * dh
                            agd = emit_partial_d(
                                li + 1, fg0, fg1, [xnext], f"l{li + 1}g{gi}"
                            )
                            if gi == 0:
                                d_pending = []
                            d_pending.append(agd)
                        elif last_group:
                            d_pending = [
                                emit_partial_d(li + 1, 0, fout, [xnext], f"l{li + 1}")
                            ]

                if debug_taps:
                    nc.sync.dma_start(dbg_x[li][:], xnext[:])
                if is_last:
                    psum_final = small.tile([fout, 1], dt.float32, tag="vec1f")
                    nc.vector.reduce_sum(
                        psum_final[:], xnext[:], axis=mybir.AxisListType.X
                    )
                    nc.sync.dma_start(pool_out[:], psum_final[:])
                else:
                    xTown_cur = [xnext]

    nc.finalize()
    return nc


_NC_CACHE = None
_last_in_maps = None


def kernel(**inputs):
    global _NC_CACHE
    node_features = np.asarray(inputs["node_features"], dtype=np.float32)
    adj = np.ascontiguousarray(np.asarray(inputs["adj_mat"], dtype=np.int32))
    fc_w = np.asarray(inputs["fc_w"], dtype=np.float32)
    fc_b = np.asarray(inputs["fc_b"], dtype=np.float32)

    x0T = node_features.T  # [256, N]

    wext = {}
    wd = {}
    ws = {}
    for li, (fin, fout, h, concat, _elu, _g) in enumerate(CFG, start=1):
        dh = fout // h if concat else fout
        W = np.asarray(inputs[f"W{li}"], dtype=np.float32)  # [h, fin, dh]
        a_src = np.asarray(inputs[f"a_src{li}"], dtype=np.float32)  # [h, dh]
        a_dst = np.asarray(inputs[f"a_dst{li}"], dtype=np.float32)
        wext[li] = np.ascontiguousarray(W.transpose(1, 0, 2).reshape(fin, h * dh))
        wd[li] = np.ascontiguousarray(np.einsum("hfd,hd->fh", W, a_dst).astype(np.float32))
        ws[li] = np.ascontiguousarray(np.einsum("hfd,hd->fh", W, a_src).astype(np.float32))

    in_maps = []
    for c in range(NCORE):
        m = {
            "adjrows": np.ascontiguousarray(adj[c * ROWS : (c + 1) * ROWS, :]),
            "x0T_own": np.ascontiguousarray(x0T[:, c * ROWS : (c + 1) * ROWS]),
        }
        for li in range(1, 6):
            m[f"wext{li}"] = wext[li]
            m[f"wd{li}"] = wd[li]
            m[f"ws{li}"] = ws[li]
        in_maps.append(m)

    if _NC_CACHE is None:
        _NC_CACHE = build_kernel()
    nc = _NC_CACHE
    global _last_in_maps
    _last_in_maps = in_maps

    res = run_bass_kernel_spmd(nc, in_maps, list(range(NCORE)))
    total = np.zeros((8,), dtype=np.float32)
    for c in range(NCORE):
        total += res.results[c]["pool_part"][:, 0]
    pooled = total / np.float32(N)
    out = pooled @ fc_w + fc_b
    return out.astype(np.float32)
